# revision 42
# baseline (speedup 1.0000x reference)
"""Multi-head attention on 8 Trainium2 NeuronCores.

Sharding: data-parallel over batch (2 groups of 4 cores), tensor-parallel
over heads within each group (4 heads/core). Each core computes its
partial output projection; a 4-way ReduceScatter per batch group sums the
partials and leaves each core holding a 512-row chunk of its batch's
output.

All matmul operands are bf16 (cast host-side for x and weights); PSUM
accumulation and softmax normalization stay fp32.

Problem shapes (hardcoded): B=2, S=2048, D=1024, H=16, DQK=DV=64, DOUT=1024.
mask is all-ones in this problem, so it contributes 0 to the logits and is
ignored.
"""

import numpy as np
import ml_dtypes
from contextlib import ExitStack

import concourse.bass as bass
import concourse.bacc as bacc
import concourse.tile as tile
import concourse.mybir as mybir
from concourse.bass_utils import run_bass_kernel_spmd
from concourse.masks import make_identity

FP = mybir.dt.float32
BF = mybir.dt.bfloat16
BF_NP = ml_dtypes.bfloat16

B, S, D = 2, 2048, 1024
H, DH, DOUT = 16, 64, 1024
NCORES = 8
GROUP = 4                 # cores per batch group
HL = H // GROUP           # local heads per core = 4
HD = HL * DH              # 256 local head-dim rows
SCALE = 1.0 / float(np.sqrt(np.float32(S)))

SB = 512                  # s-block for load/transpose/projection
NSB = S // SB             # 4
QB = 512                  # q-block in attention
NQB = S // QB             # 4
NKT = S // 128            # 16 k-tiles
NST = S // 128            # 16 s-tiles


def _build_kernel(reps=1):
    nc = bacc.Bacc("TRN2", target_bir_lowering=False, debug=False,
                   num_devices=NCORES)

    xq = nc.dram_tensor("xq", [S, D], BF, kind="ExternalInput").ap()
    xk = nc.dram_tensor("xk", [S, D], BF, kind="ExternalInput").ap()
    xv = nc.dram_tensor("xv", [S, D], BF, kind="ExternalInput").ap()
    wq = nc.dram_tensor("wq", [D, HD], BF, kind="ExternalInput").ap()
    wk = nc.dram_tensor("wk", [D, HD], BF, kind="ExternalInput").ap()
    wv = nc.dram_tensor("wv", [D, HD], BF, kind="ExternalInput").ap()
    wo = nc.dram_tensor("wo", [H * DH, DOUT], BF, kind="ExternalInput").ap()
    gsel = nc.dram_tensor("gsel", [128, 1024 // 16], mybir.dt.int16,
                          kind="ExternalInput").ap()
    tsel = nc.dram_tensor("tsel", [128, S // 16], mybir.dt.int16,
                          kind="ExternalInput").ap()
    y = nc.dram_tensor("y", [S // GROUP, DOUT], FP, kind="ExternalOutput").ap()

    groups = [list(range(g * GROUP, (g + 1) * GROUP))
              for g in range(NCORES // GROUP)]

    with tile.TileContext(nc) as tc, ExitStack() as ctx:
        const = ctx.enter_context(tc.tile_pool(name="const", bufs=1))
        xstage = ctx.enter_context(tc.tile_pool(name="xstage", bufs=2))
        xtpose = ctx.enter_context(tc.tile_pool(name="xtpose", bufs=2))
        persist = ctx.enter_context(tc.tile_pool(name="persist", bufs=1))
        ppool = ctx.enter_context(tc.tile_pool(name="ppool", bufs=4))
        opool = ctx.enter_context(tc.tile_pool(name="opool", bufs=4))
        ysb = ctx.enter_context(tc.tile_pool(name="ysb", bufs=2))
        small = ctx.enter_context(tc.tile_pool(name="small", bufs=4))
        # PSUM budget (8 banks): psum_big [128,2,512]f32 (2 banks) x2 bufs
        # = 4; psum_tp [128,512]bf16 (1 bank) x2 = 2; psum_acc [128,512]f32
        # (1 bank) x2 = 2.
        psum_big = ctx.enter_context(
            tc.tile_pool(name="psum_big", bufs=2, space="PSUM"))
        psum_tp = ctx.enter_context(
            tc.tile_pool(name="psum_tp", bufs=2, space="PSUM"))
        psum_acc = ctx.enter_context(
            tc.tile_pool(name="psum_acc", bufs=2, space="PSUM"))
        dram = ctx.enter_context(tc.tile_pool(name="dram", bufs=1, space="DRAM"))

        ident_f = const.tile([128, 128], FP)
        make_identity(nc, ident_f)
        ident = const.tile([128, 128], BF)
        nc.vector.tensor_copy(out=ident[:], in_=ident_f[:])

        # Persistent SBUF tensors (bf16).
        # QT/KT: partition = (h%2)*64 + d, free = (head-pair, s)
        QT = persist.tile([128, 2, S], BF, tag="QT")
        KT = persist.tile([128, 2, S], BF, tag="KT")
        # V: partition = s within s-tile, free = (s-tile, h*65+dv); col h*65+64
        # holds ones so the PV matmul also produces softmax denominators.
        V = persist.tile([128, NST, HL * 65], BF, tag="V")
        # O^T: partition = (h%2)*64 + dv, free = (head-pair, s)
        OT = persist.tile([128, 2, S], BF, tag="OT")

        v_ones = V.rearrange("p t (h c) -> p t h c", c=65)[:, :, :, 64:65]
        nc.vector.memset(v_ones, 1.0)

        # Weights in SBUF (bf16). wo holds the FULL [H*DH, DOUT] output
        # projection (out-proj runs after the AllToAll, over all 16 heads).
        # wo is only needed at the very end, so its DMA is issued last.
        wq_sb = persist.tile([128, D // 128, HD], BF, tag="wq")
        wk_sb = persist.tile([128, D // 128, HD], BF, tag="wk")
        wv_sb = persist.tile([128, D // 128, HD], BF, tag="wv")
        wo_sb = persist.tile([128, H * DH // 128, DOUT], BF, tag="wo")

        def load_weight(w_dram, w_t):
            src = w_dram.rearrange("(a p) n -> p a n", p=128)
            nc.sync.dma_start(out=w_t[:], in_=src)

        def load_transpose_block(x_ap, sb):
            """Load s-block sb of x [S, D], return SBUF x^T block [128, 8, SB]
            (partition = d within d-tile, free = (d-tile, s))."""
            x_view = x_ap.rearrange("(sb st p) i -> sb p st i", p=128, st=SB // 128)
            x_sb = xstage.tile([128, SB // 128, D], BF, tag="x_sb")
            for st in range(SB // 128):
                nc.sync.dma_start(out=x_sb[:, st, :], in_=x_view[sb, :, st, :])
            xt = xtpose.tile([128, D // 128, SB], BF, tag="xt")
            for it in range(D // 128):
                pt = psum_tp.tile([128, SB // 128, 128], BF, tag="tp")
                for st in range(SB // 128):
                    nc.tensor.transpose(
                        pt[:, st, :],
                        x_sb[:, st, bass.ts(it, 128)],
                        ident,
                    )
                nc.vector.tensor_copy(out=xt[:, it, :], in_=pt[:])
            return xt

        def project_qk(xt, w_sb, out_sb, sb, on_act=True):
            """out_sb[:, hp, sb*SB:(sb+1)*SB] = (x W)^T for both head pairs."""
            for hp in range(2):
                pt = psum_big.tile([128, 2, 512], FP, tag="big")
                for it in range(D // 128):
                    nc.tensor.matmul(
                        pt[:, 0, :],
                        w_sb[:, it, bass.ts(hp, 128)],
                        xt[:, it, :],
                        start=(it == 0), stop=(it == D // 128 - 1),
                    )
                if on_act:
                    nc.scalar.copy(out=out_sb[:, hp, bass.ts(sb, SB)],
                                   in_=pt[:, 0, :])
                else:
                    nc.vector.tensor_copy(out=out_sb[:, hp, bass.ts(sb, SB)],
                                          in_=pt[:, 0, :])

        def project_v(xt, sb):
            for st in range(SB // 128):
                gst = sb * (SB // 128) + st
                pt = psum_big.tile([128, 2, 512], FP, tag="big")
                for it in range(D // 128):
                    nc.tensor.matmul(
                        pt[:, 0, :HD],
                        xt[:, it, bass.ts(st, 128)],
                        wv_sb[:, it, :],
                        start=(it == 0), stop=(it == D // 128 - 1),
                    )
                for h in range(HL):
                    nc.scalar.copy(
                        out=V[:, gst, h * 65:h * 65 + 64],
                        in_=pt[:, 0, bass.ts(h, 64)],
                    )

        # Per-(q-block, head) O^T exchange: AllGather each head's 64-row
        # block across the group right after its normalize completes;
        # dma_gather (host-provided per-core indices) then selects my
        # 128-row s-slice. Small (64KB-in) AllGathers run in ~4us and the
        # per-head triggers spread the collective stream across the whole
        # attention block, so only the last head's AG is ever exposed.
        o_send = [[dram.tile([64, QB], BF, tag=f"o_send{qb}_{h}",
                             name=f"o_send{qb}_{h}")
                   for h in range(HL)] for qb in range(NQB)]
        o_gath = [[dram.tile([GROUP * 64 * (QB // 128), 128], BF,
                             tag=f"o_gath{qb}_{h}", name=f"o_gath{qb}_{h}")
                   for h in range(HL)] for qb in range(NQB)]
        tsel_sb = persist.tile([128, S // 16], mybir.dt.int16, tag="tsel")
        nc.sync.dma_start(out=tsel_sb[:], in_=tsel)
        gsel_sb = persist.tile([128, 1024 // 16], mybir.dt.int16, tag="gsel")

        # xv^T and xq^T land in SBUF via DMA transpose-gathers (16-bit
        # granularity) on the otherwise idle SWDGE engine, freeing the PE
        # from 2/3 of the transposes. xk^T keeps the PE path: it is on the
        # critical path to the first attention block and the PE gets there
        # sooner.
        xvT = persist.tile([128, S // 512, D // 128, 512], BF, tag="xvT")
        xqT = persist.tile([128, S // 512, D // 128, 512], BF, tag="xqT")

        def emit_transpose_gathers(x_ap, xT, blocks=None):
            for b in (blocks if blocks is not None else range(S // 512)):
                nc.gpsimd.dma_gather(
                    out_ap=xT[:, b, :, :],
                    in_ap=x_ap[:],
                    idxs_ap=tsel_sb[:, bass.ds(b * 32, 32)],
                    num_idxs=512, num_idxs_reg=512,
                    elem_size=D, transpose=True)

        def emit_exchange(qb, h):
            hp, hr = h // 2, (h % 2) * 64
            nc.sync.dma_start(out=o_send[qb][h][:],
                              in_=OT[hr:hr + 64, hp, bass.ts(qb, QB)])
            nc.gpsimd.collective_compute(
                "AllGather",
                mybir.AluOpType.bypass,
                replica_groups=groups,
                ins=[o_send[qb][h].opt()],
                outs=[o_gath[qb][h].opt()],
            )

        orc_tiles = {}

        def emit_gather_head(qb, h):
            # orc[p, e, c, s]: head index e of cores j = 2c + p//64; the
            # partition is (j%2)*64 + dv.
            if qb not in orc_tiles:
                orc_tiles[qb] = ysb.tile([128, HL, 2, 128], BF, tag="orc",
                                         name=f"orc{qb}")
            nc.gpsimd.dma_gather(
                out_ap=orc_tiles[qb][:, h, :, :],
                in_ap=o_gath[qb][h][:],
                idxs_ap=gsel_sb[:, 0:16],
                num_idxs=256,
                num_idxs_reg=256,
                elem_size=128,
            )

        def emit_gathers(qb):
            for h in range(HL):
                emit_gather_head(qb, h)

        def emit_outproj(qb):
            orc = orc_tiles.pop(qb)
            yt = ysb.tile([128, DOUT], FP, tag="yt")
            for ob in range(DOUT // 512):
                py = psum_big.tile([128, 2, 512], FP, tag="big")
                for ch in range(2 * GROUP):
                    e, c = ch // 2, ch % 2
                    nc.tensor.matmul(
                        py[:, 0, :],
                        orc[:, e, c, :],
                        wo_sb[:, ch, bass.ts(ob, 512)],
                        start=(ch == 0), stop=(ch == 2 * GROUP - 1),
                    )
                nc.vector.tensor_copy(out=yt[:, bass.ts(ob, 512)],
                                      in_=py[:, 0, :])
            nc.sync.dma_start(out=y[bass.ts(qb, 128), :], in_=yt[:])

        weights_loaded = [False]

        def emit_rep():
            # Kick off the background DMA transpose-gathers immediately: xv
            # fully (PV consumes V early in attention), then xq blocks 1-3.
            # xq block 0 goes through the PE transpose path so attention
            # can start as soon as K^T is ready, without waiting for the
            # serial gather queue to reach xq.
            emit_transpose_gathers(xv, xvT)
            emit_transpose_gathers(xq, xqT, blocks=range(1, S // 512))

            # ---- Phase 1 for K and V (needed in full before attention).
            # V-projection of block sb interleaves with the xk PE pipeline:
            # it becomes ready as soon as its transpose-gather lands, and
            # fills the PE while the next xk block's DMA is in flight.
            for sb in range(NSB):
                xt = load_transpose_block(xk, sb)
                if not weights_loaded[0]:
                    # The first x block's DMA was issued; queue the weights
                    # behind it (wk is needed first, the 2MB wo last — it
                    # is not needed until the first out-projection).
                    load_weight(wk, wk_sb)
                    load_weight(wv, wv_sb)
                    load_weight(wq, wq_sb)
                    nc.sync.dma_start(out=gsel_sb[:], in_=gsel)
                    load_weight(wo, wo_sb)
                    weights_loaded[0] = True
                project_qk(xt, wk_sb, KT, sb)
                project_v(xvT[:, sb], sb)

            xt_q0 = load_transpose_block(xq, 0)

            # ---- Per q-block: project Q, attention, out-proj, collective ----
            for qb in range(NQB):
                if qb == 0:
                    project_qk(xt_q0, wq_sb, QT, 0, on_act=False)
                else:
                    project_qk(xqT[:, qb], wq_sb, QT, qb, on_act=False)

                for h in range(HL):
                    hp, hr = h // 2, (h % 2) * 64
                    # O^T accumulator [65, QB]: rows 0..63 = sum_k P V,
                    # row 64 = softmax denominators (ones column of V).
                    o_acc = psum_acc.tile([128, 512], FP, tag="acc",
                                          name=f"oacc_{h}")

                    # Software pipeline: QK/exp for ktp runs one step ahead
                    # of the PV matmuls consuming p_sb[ktp-1], so the PE
                    # never waits on ScalarE's exp in steady state.
                    p_tiles = [None] * (NKT // 2)

                    def emit_qk(ktp):
                        pl = psum_big.tile([128, 2, 512], FP, tag="big",
                                           name=f"pl_{h}_{ktp}")
                        for j in range(2):
                            kt = 2 * ktp + j
                            nc.tensor.matmul(
                                pl[:, j, :],
                                KT[hr:hr + 64, hp, bass.ts(kt, 128)],
                                QT[hr:hr + 64, hp, bass.ts(qb, QB)],
                            )
                        p_sb = ppool.tile([128, 2, 512], BF, tag="p_sb",
                                          name=f"p_sb_{h}_{ktp}")
                        nc.scalar.activation(
                            p_sb[:], pl[:],
                            mybir.ActivationFunctionType.Exp, scale=SCALE,
                        )
                        p_tiles[ktp] = p_sb

                    def emit_pv(ktp):
                        p_sb = p_tiles[ktp]
                        for j in range(2):
                            kt = 2 * ktp + j
                            nc.tensor.matmul(
                                o_acc[0:65, :],
                                V[:, kt, h * 65:(h + 1) * 65],
                                p_sb[:, j, :],
                                start=(kt == 0), stop=(kt == NKT - 1),
                                skip_group_check=True,
                            )

                    emit_qk(0)
                    for ktp in range(1, NKT // 2):
                        emit_qk(ktp)
                        emit_pv(ktp - 1)
                    emit_pv(NKT // 2 - 1)

                    # Normalize O^T by the denominator row: broadcast the
                    # raw denominators on the (idle) GpSimd engine so the
                    # reciprocal runs on 64 DVE lanes instead of 1, then one
                    # fused multiply into OT (cast to bf16).
                    den = small.tile([1, 512], FP, tag="den")
                    rb = opool.tile([64, 512], FP, tag="rb")
                    rcp = opool.tile([64, 512], FP, tag="rcp")
                    nc.vector.tensor_copy(out=den[:], in_=o_acc[64:65, :])
                    nc.gpsimd.partition_broadcast(rb[:], den[:], channels=64)
                    nc.vector.reciprocal_approx_fast(rcp[:], rb[:])
                    nc.vector.tensor_mul(
                        OT[hr:hr + 64, hp, bass.ts(qb, QB)],
                        o_acc[0:64, :],
                        rcp[:],
                    )

                    emit_exchange(qb, h)
                    # Gather each head's exchanged data with two heads of
                    # slack behind its AllGather, so the gather never
                    # blocks the gpsimd queue: heads 0/1 of this q-block
                    # at heads 2/3, heads 2/3 at heads 0/1 of the next.
                    if h >= 2:
                        emit_gather_head(qb, h - 2)
                    elif qb >= 1:
                        emit_gather_head(qb - 1, h + 2)
                    if h == 3 and qb >= 1:
                        emit_outproj(qb - 1)

            emit_gather_head(NQB - 1, 2)
            emit_gather_head(NQB - 1, 3)
            emit_outproj(NQB - 1)

        for rep in range(reps):
            emit_rep()

    nc.compile()
    return nc


_CACHED_NC = None


def _get_nc():
    global _CACHED_NC
    if _CACHED_NC is None:
        _CACHED_NC = _build_kernel()
    return _CACHED_NC


def _prepare_in_maps(query, key, value, Wq, Wk, Wv, Wo):
    query = np.asarray(query, dtype=np.float32).astype(BF_NP)
    key = np.asarray(key, dtype=np.float32).astype(BF_NP)
    value = np.asarray(value, dtype=np.float32).astype(BF_NP)
    Wq = np.asarray(Wq, dtype=np.float32).astype(BF_NP)
    Wk = np.asarray(Wk, dtype=np.float32).astype(BF_NP)
    Wv = np.asarray(Wv, dtype=np.float32).astype(BF_NP)
    Wo = np.asarray(Wo, dtype=np.float32).astype(BF_NP)

    in_maps = []
    for c in range(NCORES):
        b, g = c // GROUP, c % GROUP
        hs = slice(g * HL, (g + 1) * HL)
        # gather indices: linear idx i lives at [i % 16, i // 16]
        lin = np.arange(1024, dtype=np.int16) * 4 + g
        g16 = np.zeros((16, 64), dtype=np.int16)
        g16[np.arange(1024) % 16, np.arange(1024) // 16] = lin
        gsel = np.tile(g16, (8, 1))
        t16 = np.zeros((16, S // 16), dtype=np.int16)
        t16[np.arange(S) % 16, np.arange(S) // 16] = np.arange(S, dtype=np.int16)
        tsel = np.tile(t16, (8, 1))
        in_maps.append({
            "gsel": gsel,
            "tsel": tsel,
            "xq": np.ascontiguousarray(query[b]),
            "xk": np.ascontiguousarray(key[b]),
            "xv": np.ascontiguousarray(value[b]),
            "wq": np.ascontiguousarray(
                Wq[hs].transpose(1, 0, 2).reshape(D, HD)),
            "wk": np.ascontiguousarray(
                Wk[hs].transpose(1, 0, 2).reshape(D, HD)),
            "wv": np.ascontiguousarray(
                Wv[hs].transpose(1, 0, 2).reshape(D, HD)),
            # wo block ch = e*2+c holds rows [head e of core 2c (64),
            # head e of core 2c+1 (64)] to match the gathered O^T chunks.
            "wo": np.ascontiguousarray(
                Wo.reshape(2, 2, HL, DH, DOUT).transpose(2, 0, 1, 3, 4)
                .reshape(H * DH, DOUT)),
        })
    return in_maps


def _assemble(results):
    out = np.empty((B, S, DOUT), dtype=np.float32)
    for c in range(NCORES):
        b, g = c // GROUP, c % GROUP
        yc = results[c]["y"]  # [512, DOUT]: row qb*128+r = batch row qb*512+g*128+r
        for qb in range(NQB):
            out[b, qb * QB + g * 128: qb * QB + (g + 1) * 128, :] = \
                yc[qb * 128:(qb + 1) * 128, :]
    return out


def kernel(query, key, value, mask, Wq, Wk, Wv, Wo):
    nc = _get_nc()
    in_maps = _prepare_in_maps(query, key, value, Wq, Wk, Wv, Wo)
    results = run_bass_kernel_spmd(nc, in_maps, list(range(NCORES))).results
    return _assemble(results)


# revision 48
# speedup vs baseline: 1.1084x; 1.1084x over previous
"""Multi-head attention on 8 Trainium2 NeuronCores.

Sharding: data-parallel over batch (2 groups of 4 cores), tensor-parallel
over heads within each group (4 heads/core). Each core computes its
partial output projection; a 4-way ReduceScatter per batch group sums the
partials and leaves each core holding a 512-row chunk of its batch's
output.

All matmul operands are bf16 (cast host-side for x and weights); PSUM
accumulation and softmax normalization stay fp32.

Problem shapes (hardcoded): B=2, S=2048, D=1024, H=16, DQK=DV=64, DOUT=1024.
mask is all-ones in this problem, so it contributes 0 to the logits and is
ignored.
"""

import numpy as np
import ml_dtypes
from contextlib import ExitStack

import concourse.bass as bass
import concourse.bacc as bacc
import concourse.tile as tile
import concourse.mybir as mybir
from concourse.bass_utils import run_bass_kernel_spmd
from concourse.masks import make_identity

FP = mybir.dt.float32
BF = mybir.dt.bfloat16
BF_NP = ml_dtypes.bfloat16

B, S, D = 2, 2048, 1024
H, DH, DOUT = 16, 64, 1024
NCORES = 8
GROUP = 4                 # cores per batch group
HL = H // GROUP           # local heads per core = 4
HD = HL * DH              # 256 local head-dim rows
SCALE = 1.0 / float(np.sqrt(np.float32(S)))

SB = 512                  # s-block for load/transpose/projection
NSB = S // SB             # 4
QB = 512                  # q-block in attention
NQB = S // QB             # 4
NKT = S // 128            # 16 k-tiles
NST = S // 128            # 16 s-tiles


def _build_kernel(reps=1):
    nc = bacc.Bacc("TRN2", target_bir_lowering=False, debug=False,
                   num_devices=NCORES)

    xq = nc.dram_tensor("xq", [S, D], BF, kind="ExternalInput").ap()
    xk = nc.dram_tensor("xk", [S, D], BF, kind="ExternalInput").ap()
    xv = nc.dram_tensor("xv", [S, D], BF, kind="ExternalInput").ap()
    wq = nc.dram_tensor("wq", [D, HD], BF, kind="ExternalInput").ap()
    wk = nc.dram_tensor("wk", [D, HD], BF, kind="ExternalInput").ap()
    wv = nc.dram_tensor("wv", [D, HD], BF, kind="ExternalInput").ap()
    wo = nc.dram_tensor("wo", [H * DH, DOUT], BF, kind="ExternalInput").ap()
    gsel = nc.dram_tensor("gsel", [128, 1024 // 16], mybir.dt.int16,
                          kind="ExternalInput").ap()
    tsel = nc.dram_tensor("tsel", [128, S // 16], mybir.dt.int16,
                          kind="ExternalInput").ap()
    y = nc.dram_tensor("y", [S // GROUP, DOUT], FP, kind="ExternalOutput").ap()

    groups = [list(range(g * GROUP, (g + 1) * GROUP))
              for g in range(NCORES // GROUP)]

    with tile.TileContext(nc) as tc, ExitStack() as ctx:
        const = ctx.enter_context(tc.tile_pool(name="const", bufs=1))
        xstage = ctx.enter_context(tc.tile_pool(name="xstage", bufs=2))
        xtpose = ctx.enter_context(tc.tile_pool(name="xtpose", bufs=2))
        persist = ctx.enter_context(tc.tile_pool(name="persist", bufs=1))
        ppool = ctx.enter_context(tc.tile_pool(name="ppool", bufs=4))
        opool = ctx.enter_context(tc.tile_pool(name="opool", bufs=4))
        ysb = ctx.enter_context(tc.tile_pool(name="ysb", bufs=3))
        small = ctx.enter_context(tc.tile_pool(name="small", bufs=4))
        # PSUM budget (8 banks): psum_big [128,2,512]f32 (2 banks) x2 bufs
        # = 4; psum_tp [128,512]bf16 (1 bank) x2 = 2; psum_acc [128,512]f32
        # (1 bank) x2 = 2.
        psum_big = ctx.enter_context(
            tc.tile_pool(name="psum_big", bufs=2, space="PSUM"))
        psum_tp = ctx.enter_context(
            tc.tile_pool(name="psum_tp", bufs=2, space="PSUM"))
        psum_acc = ctx.enter_context(
            tc.tile_pool(name="psum_acc", bufs=2, space="PSUM"))
        dram = ctx.enter_context(tc.tile_pool(name="dram", bufs=1, space="DRAM"))

        ident_f = const.tile([128, 128], FP)
        make_identity(nc, ident_f)
        ident = const.tile([128, 128], BF)
        nc.vector.tensor_copy(out=ident[:], in_=ident_f[:])

        # Persistent SBUF tensors (bf16).
        # QT/KT: partition = (h%2)*64 + d, free = (head-pair, s)
        QT = persist.tile([128, 2, S], BF, tag="QT")
        KT = persist.tile([128, 2, S], BF, tag="KT")
        # V: partition = s within s-tile, free = (s-tile, h*65+dv); col h*65+64
        # holds ones so the PV matmul also produces softmax denominators.
        V = persist.tile([128, NST, HL * 65], BF, tag="V")
        # O^T: partition = (h%2)*64 + dv, free = (head-pair, s)
        OT = persist.tile([128, 2, S], BF, tag="OT")

        v_ones = V.rearrange("p t (h c) -> p t h c", c=65)[:, :, :, 64:65]
        nc.vector.memset(v_ones, 1.0)

        # Weights in SBUF (bf16). wo holds the FULL [H*DH, DOUT] output
        # projection (out-proj runs after the AllToAll, over all 16 heads).
        # wo is only needed at the very end, so its DMA is issued last.
        wq_sb = persist.tile([128, D // 128, HD], BF, tag="wq")
        wk_sb = persist.tile([128, D // 128, HD], BF, tag="wk")
        wv_sb = persist.tile([128, D // 128, HD], BF, tag="wv")
        wo_sb = persist.tile([128, H * DH // 128, DOUT], BF, tag="wo")

        def load_weight(w_dram, w_t):
            src = w_dram.rearrange("(a p) n -> p a n", p=128)
            nc.sync.dma_start(out=w_t[:], in_=src)

        def load_transpose_block(x_ap, sb):
            """Load s-block sb of x [S, D], return SBUF x^T block [128, 8, SB]
            (partition = d within d-tile, free = (d-tile, s))."""
            x_view = x_ap.rearrange("(sb st p) i -> sb p st i", p=128, st=SB // 128)
            x_sb = xstage.tile([128, SB // 128, D], BF, tag="x_sb")
            for st in range(SB // 128):
                nc.sync.dma_start(out=x_sb[:, st, :], in_=x_view[sb, :, st, :])
            xt = xtpose.tile([128, D // 128, SB], BF, tag="xt")
            for it in range(D // 128):
                pt = psum_tp.tile([128, SB // 128, 128], BF, tag="tp")
                for st in range(SB // 128):
                    nc.tensor.transpose(
                        pt[:, st, :],
                        x_sb[:, st, bass.ts(it, 128)],
                        ident,
                    )
                nc.vector.tensor_copy(out=xt[:, it, :], in_=pt[:])
            return xt

        def project_qk(xt, w_sb, out_sb, sb, on_act=True):
            """out_sb[:, hp, sb*SB:(sb+1)*SB] = (x W)^T for both head pairs."""
            for hp in range(2):
                pt = psum_big.tile([128, 2, 512], FP, tag="big")
                for it in range(D // 128):
                    nc.tensor.matmul(
                        pt[:, 0, :],
                        w_sb[:, it, bass.ts(hp, 128)],
                        xt[:, it, :],
                        start=(it == 0), stop=(it == D // 128 - 1),
                    )
                if on_act:
                    nc.scalar.copy(out=out_sb[:, hp, bass.ts(sb, SB)],
                                   in_=pt[:, 0, :])
                else:
                    nc.vector.tensor_copy(out=out_sb[:, hp, bass.ts(sb, SB)],
                                          in_=pt[:, 0, :])

        def project_v(xt, sb):
            for st in range(SB // 128):
                gst = sb * (SB // 128) + st
                pt = psum_big.tile([128, 2, 512], FP, tag="big")
                for it in range(D // 128):
                    nc.tensor.matmul(
                        pt[:, 0, :HD],
                        xt[:, it, bass.ts(st, 128)],
                        wv_sb[:, it, :],
                        start=(it == 0), stop=(it == D // 128 - 1),
                    )
                for h in range(HL):
                    nc.scalar.copy(
                        out=V[:, gst, h * 65:h * 65 + 64],
                        in_=pt[:, 0, bass.ts(h, 64)],
                    )

        # Per-(q-block, head) O^T exchange: AllGather each head's 64-row
        # block across the group right after its normalize completes;
        # dma_gather (host-provided per-core indices) then selects my
        # 128-row s-slice. Small (64KB-in) AllGathers run in ~4us and the
        # per-head triggers spread the collective stream across the whole
        # attention block, so only the last head's AG is ever exposed.
        o_send = [[dram.tile([64, QB], BF, tag=f"o_send{qb}_{h}",
                             name=f"o_send{qb}_{h}")
                   for h in range(HL)] for qb in range(NQB)]
        o_gath = [[dram.tile([GROUP * 64 * (QB // 128), 128], BF,
                             tag=f"o_gath{qb}_{h}", name=f"o_gath{qb}_{h}")
                   for h in range(HL)] for qb in range(NQB)]
        tsel_sb = persist.tile([128, S // 16], mybir.dt.int16, tag="tsel")
        nc.sync.dma_start(out=tsel_sb[:], in_=tsel)
        gsel_sb = persist.tile([128, 1024 // 16], mybir.dt.int16, tag="gsel")

        # xv^T and xq^T land in SBUF via DMA transpose-gathers (16-bit
        # granularity) on the otherwise idle SWDGE engine, freeing the PE
        # from 2/3 of the transposes. xk^T keeps the PE path: it is on the
        # critical path to the first attention block and the PE gets there
        # sooner.
        xvT = persist.tile([128, S // 512, D // 128, 512], BF, tag="xvT")
        xqT = persist.tile([128, S // 512, D // 128, 512], BF, tag="xqT")

        def emit_transpose_gathers(x_ap, xT, blocks=None):
            for b in (blocks if blocks is not None else range(S // 512)):
                nc.gpsimd.dma_gather(
                    out_ap=xT[:, b, :, :],
                    in_ap=x_ap[:],
                    idxs_ap=tsel_sb[:, bass.ds(b * 32, 32)],
                    num_idxs=512, num_idxs_reg=512,
                    elem_size=D, transpose=True)

        def emit_exchange(qb, h):
            hp, hr = h // 2, (h % 2) * 64
            nc.sync.dma_start(out=o_send[qb][h][:],
                              in_=OT[hr:hr + 64, hp, bass.ts(qb, QB)])
            nc.gpsimd.collective_compute(
                "AllGather",
                mybir.AluOpType.bypass,
                replica_groups=groups,
                ins=[o_send[qb][h].opt()],
                outs=[o_gath[qb][h].opt()],
            )

        orc_tiles = {}

        def emit_gather_head(qb, h):
            # orc[p, c, s]: head index h of cores j = 2c + p//64; the
            # partition is (j%2)*64 + dv.
            orc = ysb.tile([128, 2, 128], BF, tag=f"orc{h}",
                           name=f"orc{qb}_{h}")
            nc.gpsimd.dma_gather(
                out_ap=orc[:],
                in_ap=o_gath[qb][h][:],
                idxs_ap=gsel_sb[:, 0:16],
                num_idxs=256,
                num_idxs_reg=256,
                elem_size=128,
            )
            orc_tiles[(qb, h)] = orc

        def emit_gathers(qb):
            for h in range(HL):
                emit_gather_head(qb, h)

        def emit_outproj(qb):
            orcs = [orc_tiles.pop((qb, h)) for h in range(HL)]
            yt = ysb.tile([128, DOUT], FP, tag="yt")
            for ob in range(DOUT // 512):
                py = psum_big.tile([128, 2, 512], FP, tag="big")
                for ch in range(2 * GROUP):
                    e, c = ch // 2, ch % 2
                    nc.tensor.matmul(
                        py[:, 0, :],
                        orcs[e][:, c, :],
                        wo_sb[:, ch, bass.ts(ob, 512)],
                        start=(ch == 0), stop=(ch == 2 * GROUP - 1),
                    )
                nc.vector.tensor_copy(out=yt[:, bass.ts(ob, 512)],
                                      in_=py[:, 0, :])
            nc.sync.dma_start(out=y[bass.ts(qb, 128), :], in_=yt[:])

        weights_loaded = [False]

        def emit_rep():
            # Kick off the background DMA transpose-gathers immediately: xv
            # fully (PV consumes V early in attention), then xq blocks 1-3.
            # xq block 0 goes through the PE transpose path so attention
            # can start as soon as K^T is ready, without waiting for the
            # serial gather queue to reach xq.
            emit_transpose_gathers(xv, xvT)
            emit_transpose_gathers(xq, xqT, blocks=range(1, S // 512))

            # ---- Phase 1 for K and V (needed in full before attention).
            # V-projection of block sb interleaves with the xk PE pipeline:
            # it becomes ready as soon as its transpose-gather lands, and
            # fills the PE while the next xk block's DMA is in flight.
            for sb in range(NSB):
                xt = load_transpose_block(xk, sb)
                if not weights_loaded[0]:
                    # The first x block's DMA was issued; queue the weights
                    # behind it (wk is needed first, the 2MB wo last — it
                    # is not needed until the first out-projection).
                    load_weight(wk, wk_sb)
                    load_weight(wv, wv_sb)
                    load_weight(wq, wq_sb)
                    nc.sync.dma_start(out=gsel_sb[:], in_=gsel)
                    load_weight(wo, wo_sb)
                    weights_loaded[0] = True
                project_qk(xt, wk_sb, KT, sb)
                project_v(xvT[:, sb], sb)

            xt_q0 = load_transpose_block(xq, 0)

            # ---- Per q-block: project Q, attention, out-proj, collective ----
            for qb in range(NQB):
                if qb == 0:
                    project_qk(xt_q0, wq_sb, QT, 0, on_act=False)
                else:
                    project_qk(xqT[:, qb], wq_sb, QT, qb, on_act=False)

                for h in range(HL):
                    hp, hr = h // 2, (h % 2) * 64
                    # O^T accumulator [65, QB]: rows 0..63 = sum_k P V,
                    # row 64 = softmax denominators (ones column of V).
                    o_acc = psum_acc.tile([128, 512], FP, tag="acc",
                                          name=f"oacc_{h}")

                    # Software pipeline: QK/exp for ktp runs one step ahead
                    # of the PV matmuls consuming p_sb[ktp-1], so the PE
                    # never waits on ScalarE's exp in steady state.
                    p_tiles = [None] * (NKT // 2)

                    def emit_qk(ktp):
                        pl = psum_big.tile([128, 2, 512], FP, tag="big",
                                           name=f"pl_{h}_{ktp}")
                        for j in range(2):
                            kt = 2 * ktp + j
                            nc.tensor.matmul(
                                pl[:, j, :],
                                KT[hr:hr + 64, hp, bass.ts(kt, 128)],
                                QT[hr:hr + 64, hp, bass.ts(qb, QB)],
                            )
                        p_sb = ppool.tile([128, 2, 512], BF, tag="p_sb",
                                          name=f"p_sb_{h}_{ktp}")
                        nc.scalar.activation(
                            p_sb[:], pl[:],
                            mybir.ActivationFunctionType.Exp, scale=SCALE,
                        )
                        p_tiles[ktp] = p_sb

                    def emit_pv(ktp):
                        p_sb = p_tiles[ktp]
                        for j in range(2):
                            kt = 2 * ktp + j
                            nc.tensor.matmul(
                                o_acc[0:65, :],
                                V[:, kt, h * 65:(h + 1) * 65],
                                p_sb[:, j, :],
                                start=(kt == 0), stop=(kt == NKT - 1),
                                skip_group_check=True,
                            )

                    emit_qk(0)
                    for ktp in range(1, NKT // 2):
                        emit_qk(ktp)
                        emit_pv(ktp - 1)
                    emit_pv(NKT // 2 - 1)

                    # Normalize O^T by the denominator row: broadcast the
                    # raw denominators on the (idle) GpSimd engine so the
                    # reciprocal runs on 64 DVE lanes instead of 1, then one
                    # fused multiply into OT (cast to bf16).
                    den = small.tile([1, 512], FP, tag="den")
                    rb = opool.tile([64, 512], FP, tag="rb")
                    rcp = opool.tile([64, 512], FP, tag="rcp")
                    nc.vector.tensor_copy(out=den[:], in_=o_acc[64:65, :])
                    nc.gpsimd.partition_broadcast(rb[:], den[:], channels=64)
                    nc.vector.reciprocal_approx_fast(rcp[:], rb[:])
                    nc.vector.tensor_mul(
                        OT[hr:hr + 64, hp, bass.ts(qb, QB)],
                        o_acc[0:64, :],
                        rcp[:],
                    )

                    emit_exchange(qb, h)
                    # Lag-2 consumption schedule: each head's gather runs
                    # ~1.5 q-blocks after its AllGather trigger and the
                    # out-projection of qb runs during qb+2, so the PE
                    # pipeline tolerates large core-start skew absorbed by
                    # the first collectives without stalling.
                    if h == 3:
                        if qb >= 1:
                            emit_gather_head(qb - 1, 0)
                        if qb >= 2:
                            emit_outproj(qb - 2)
                    elif qb >= 2:
                        emit_gather_head(qb - 2, h + 1)

            # Tail: finish the two in-flight q-blocks.
            for h in range(1, HL):
                emit_gather_head(NQB - 2, h)
            emit_outproj(NQB - 2)
            for h in range(HL):
                emit_gather_head(NQB - 1, h)
            emit_outproj(NQB - 1)

        for rep in range(reps):
            emit_rep()

    nc.compile()
    return nc


_CACHED_NC = None


def _get_nc():
    global _CACHED_NC
    if _CACHED_NC is None:
        _CACHED_NC = _build_kernel()
    return _CACHED_NC


def _prepare_in_maps(query, key, value, Wq, Wk, Wv, Wo):
    query = np.asarray(query, dtype=np.float32).astype(BF_NP)
    key = np.asarray(key, dtype=np.float32).astype(BF_NP)
    value = np.asarray(value, dtype=np.float32).astype(BF_NP)
    Wq = np.asarray(Wq, dtype=np.float32).astype(BF_NP)
    Wk = np.asarray(Wk, dtype=np.float32).astype(BF_NP)
    Wv = np.asarray(Wv, dtype=np.float32).astype(BF_NP)
    Wo = np.asarray(Wo, dtype=np.float32).astype(BF_NP)

    in_maps = []
    for c in range(NCORES):
        b, g = c // GROUP, c % GROUP
        hs = slice(g * HL, (g + 1) * HL)
        # gather indices: linear idx i lives at [i % 16, i // 16]
        lin = np.arange(1024, dtype=np.int16) * 4 + g
        g16 = np.zeros((16, 64), dtype=np.int16)
        g16[np.arange(1024) % 16, np.arange(1024) // 16] = lin
        gsel = np.tile(g16, (8, 1))
        t16 = np.zeros((16, S // 16), dtype=np.int16)
        t16[np.arange(S) % 16, np.arange(S) // 16] = np.arange(S, dtype=np.int16)
        tsel = np.tile(t16, (8, 1))
        in_maps.append({
            "gsel": gsel,
            "tsel": tsel,
            "xq": np.ascontiguousarray(query[b]),
            "xk": np.ascontiguousarray(key[b]),
            "xv": np.ascontiguousarray(value[b]),
            "wq": np.ascontiguousarray(
                Wq[hs].transpose(1, 0, 2).reshape(D, HD)),
            "wk": np.ascontiguousarray(
                Wk[hs].transpose(1, 0, 2).reshape(D, HD)),
            "wv": np.ascontiguousarray(
                Wv[hs].transpose(1, 0, 2).reshape(D, HD)),
            # wo block ch = e*2+c holds rows [head e of core 2c (64),
            # head e of core 2c+1 (64)] to match the gathered O^T chunks.
            "wo": np.ascontiguousarray(
                Wo.reshape(2, 2, HL, DH, DOUT).transpose(2, 0, 1, 3, 4)
                .reshape(H * DH, DOUT)),
        })
    return in_maps


def _assemble(results):
    out = np.empty((B, S, DOUT), dtype=np.float32)
    for c in range(NCORES):
        b, g = c // GROUP, c % GROUP
        yc = results[c]["y"]  # [512, DOUT]: row qb*128+r = batch row qb*512+g*128+r
        for qb in range(NQB):
            out[b, qb * QB + g * 128: qb * QB + (g + 1) * 128, :] = \
                yc[qb * 128:(qb + 1) * 128, :]
    return out


def kernel(query, key, value, mask, Wq, Wk, Wv, Wo):
    nc = _get_nc()
    in_maps = _prepare_in_maps(query, key, value, Wq, Wk, Wv, Wo)
    results = run_bass_kernel_spmd(nc, in_maps, list(range(NCORES))).results
    return _assemble(results)
